# revision 2
# baseline (speedup 1.0000x reference)
"""HardMiningLoss Trainium2 kernel.

Strategy (8 NeuronCores, data-parallel over anchor-row blocks):
  - Host: stable-sort rows by class (512 classes x 16 rows), cast to bf16.
    For core r, rotate rows by -1024*r so that core's 1024 anchors sit at
    column offset 0 -> the own-class 128-col "stripe" of local anchor tile a
    is always at columns [128a, 128a+128): fully static program, identical
    NEFF on all 8 cores, per-core behaviour comes only from input data.
  - Device per core: for each of 8 anchor tiles [128 x 8192]:
      PE   : 16 bf16 matmuls (fp32 PSUM) -> sim tile
      ACT  : PSUM -> SBUF evacuation with bf16 cast (4 x 2048-wide copies)
      DVE  : tiny stripe ops (save stripe, +/-1e30 block-diagonal masking,
             min_pos via tensor_scalar accum) then three full-row passes at
             4x bf16 mode, each a single tensor_scalar with accum_out:
               max_neg   = accum-max over masked row
               neg_cnt   = accum-sum of is_gt(sim, a_thr)
               summax    = accum-sum of max(sim, a_thr)
    Per-row stats + raw 128-wide stripes are DMA'd out.
  - Host: reconstructs neg_sum = summax - a_thr*(8192-cnt), computes the
    pos side (<=15 values/row, from the stripes), loss/prec/last-row means.
"""

import numpy as np
import ml_dtypes

N = 8192
D = 128
NCLS = 512
PER = 16            # rows per class (8192/512)
MARGIN = np.float32(0.1)
NCORES = 8
RPC = N // NCORES   # rows per core = 1024
TILES = RPC // 128  # anchor tiles per core = 8
BIG = np.float32(1e30)

_BF16 = ml_dtypes.bfloat16

_compiled = {}


def _build_nc(a_star: int):
    """Build the (identical-across-cores) bass program.

    a_star: local tile index on which the extra row-sum pass runs (the tile
    that, on its owning core, contains the original last row)."""
    from contextlib import ExitStack
    import concourse.bacc as bacc
    import concourse.tile as tile
    import concourse.mybir as mybir

    dt = mybir.dt
    Alu = mybir.AluOpType

    nc = bacc.Bacc(
        "TRN2",
        debug=False,
        enable_asserts=False,
        target_bir_lowering=False,
        num_devices=NCORES,
    )

    xt_d = nc.dram_tensor("xt", [128, N], dt.bfloat16, kind="ExternalInput")
    aneg_d = nc.dram_tensor("aneg", [128, 128], dt.bfloat16, kind="ExternalInput")
    apos_d = nc.dram_tensor("apos", [128, 128], dt.bfloat16, kind="ExternalInput")
    stats_d = nc.dram_tensor("stats", [TILES, 128, 8], dt.float32, kind="ExternalOutput")
    stripes_d = nc.dram_tensor("stripes", [TILES, 128, 128], dt.bfloat16, kind="ExternalOutput")

    with tile.TileContext(nc) as tc, ExitStack() as ctx:
        xtp = ctx.enter_context(tc.tile_pool(name="xtp", bufs=1))
        cstp = ctx.enter_context(tc.tile_pool(name="cstp", bufs=1))
        sp = ctx.enter_context(tc.tile_pool(name="sp", bufs=2))
        pp = ctx.enter_context(tc.tile_pool(name="pp", bufs=2, space="PSUM"))
        trp = ctx.enter_context(tc.tile_pool(name="trp", bufs=1))
        smp = ctx.enter_context(tc.tile_pool(name="smp", bufs=3))

        xt = xtp.tile([128, N], dt.bfloat16)
        for q in range(4):
            nc.sync.dma_start(out=xt[:, q * 2048:(q + 1) * 2048],
                              in_=xt_d[:, q * 2048:(q + 1) * 2048])
        aneg = cstp.tile([128, 128], dt.bfloat16)
        apos = cstp.tile([128, 128], dt.bfloat16)
        nc.sync.dma_start(out=aneg[:], in_=aneg_d[:, :])
        nc.sync.dma_start(out=apos[:], in_=apos_d[:, :])

        trash = trp.tile([128, N], dt.bfloat16)

        for a in range(TILES):
            s = sp.tile([128, N], dt.bfloat16, tag="s")
            lhsT = xt[:, a * 128:(a + 1) * 128]
            for q in range(4):
                ps = pp.tile([128, 2048], dt.float32, tag="ps")
                for c in range(4):
                    col = q * 2048 + c * 512
                    nc.tensor.matmul(
                        ps[:, c * 512:(c + 1) * 512],
                        lhsT,
                        xt[:, col:col + 512],
                        start=True, stop=True,
                    )
                nc.scalar.copy(out=s[:, q * 2048:(q + 1) * 2048], in_=ps[:])

            stats = smp.tile([128, 8], mybir.dt.float32, tag="stats")
            nc.vector.memset(stats[:], 0.0)

            sl = slice(a * 128, (a + 1) * 128)
            ssv = smp.tile([128, 128], dt.bfloat16, tag="ssv")
            spos = smp.tile([128, 128], dt.bfloat16, tag="spos")
            nc.vector.tensor_copy(ssv[:], s[:, sl])
            nc.sync.dma_start(out=stripes_d[a], in_=ssv[:])
            nc.vector.tensor_tensor(spos[:], ssv[:], apos[:], Alu.add)

            if a == a_star:
                # full-row sums excluding the stripe (pre-masking safe:
                # regions are disjoint from the stripe).
                if a > 0:
                    nc.vector.tensor_scalar(
                        trash[:, :a * 128], s[:, :a * 128], 0.0, None,
                        Alu.add, Alu.add, accum_out=stats[:, 4:5])
                nc.vector.tensor_scalar(
                    trash[:, (a + 1) * 128:], s[:, (a + 1) * 128:], 0.0, None,
                    Alu.add, Alu.add, accum_out=stats[:, 5:6])

            # overwrite own-class stripe in s with -1e30 on the block diagonal
            nc.vector.tensor_tensor(s[:, sl], ssv[:], aneg[:], Alu.add)

            # min_pos over (own block minus self) via +1e30 masking
            nc.vector.tensor_scalar(
                trash[:, :128], spos[:], 0.0, None,
                Alu.add, Alu.min, accum_out=stats[:, 0:1])
            athr = smp.tile([128, 1], mybir.dt.float32, tag="athr")
            nc.vector.tensor_scalar_add(athr[:], stats[:, 0:1], -0.1)

            # ---- three full-row passes (bf16 4x tensor_scalar + accum) ----
            nc.vector.tensor_scalar(
                trash[:], s[:], 0.0, None,
                Alu.add, Alu.max, accum_out=stats[:, 1:2])       # max_neg
            nc.vector.tensor_scalar(
                trash[:], s[:], athr[:], None,
                Alu.is_gt, Alu.add, accum_out=stats[:, 2:3])     # neg_cnt
            nc.vector.tensor_scalar(
                trash[:], s[:], athr[:], None,
                Alu.max, Alu.add, accum_out=stats[:, 3:4])       # sum max(x,a)

            nc.sync.dma_start(out=stats_d[a], in_=stats[:])

    nc.compile()
    return nc


def _host_prep(inputs, targets):
    perm = np.argsort(targets, kind="stable")
    q_last = int(np.nonzero(perm == (N - 1))[0][0])
    Xs = np.asarray(inputs, dtype=np.float32)[perm]
    Xb = Xs.astype(_BF16)

    p = np.arange(128)
    blk_eq = (p[:, None] // PER) == (p[None, :] // PER)
    aneg = np.where(blk_eq, -BIG, np.float32(0)).astype(_BF16)
    apos = np.where(~blk_eq | np.eye(128, dtype=bool), BIG, np.float32(0)).astype(_BF16)

    in_maps = []
    for r in range(NCORES):
        xrot = np.roll(Xb, -RPC * r, axis=0)
        in_maps.append({
            "xt": np.ascontiguousarray(xrot.T),
            "aneg": aneg,
            "apos": apos,
        })
    return perm, q_last, in_maps


def _assemble(results, perm, q_last):
    """results: list (per core) of dicts with 'stats' [8,128,8] f32 and
    'stripes' [8,128,128] bf16. Returns the 4 reference outputs."""
    stats = np.stack([np.asarray(res["stats"], dtype=np.float32)
                      for res in results])                       # [8,8,128,8]
    stripes = np.stack([np.asarray(res["stripes"]).astype(np.float32)
                        for res in results])                     # [8,8,128,128]

    st = stats.reshape(N, 8)          # sorted-row order
    sv = stripes.reshape(N, 128)

    minpos = st[:, 0]
    maxneg = st[:, 1]
    cnt = st[:, 2]
    summax = st[:, 3]

    athr = minpos + np.float32(-0.1)
    neg_sum = summax - athr * (np.float32(N) - cnt)
    neg_loss = neg_sum / np.maximum(cnt, 1.0)
    valid = cnt >= 1.0

    # pos side from stripes
    p = np.arange(N) % 128
    blk = (p // PER) * PER
    own = sv[np.arange(N)[:, None], blk[:, None] + np.arange(PER)[None, :]]  # [N,16]
    self_idx = p % PER
    mask_self = np.ones((N, PER), dtype=bool)
    mask_self[np.arange(N), self_idx] = False
    pos_vals = own[mask_self].reshape(N, PER - 1)

    b = maxneg + MARGIN
    possel = pos_vals < b[:, None]
    pos_cnt = possel.sum(axis=1)
    pos_sum = np.where(possel, 1.0 - pos_vals, 0.0).sum(axis=1)
    pos_loss = pos_sum / np.maximum(pos_cnt, 1)

    loss = np.where(valid, pos_loss + neg_loss, 0.0).sum() / N
    prec = np.mean(1.0 - valid.astype(np.float32))

    # last-row means (sorted position q_last)
    i = q_last
    row_sum_excl_stripe = st[i, 4] + st[i, 5]
    stripe_sum = sv[i].sum()
    own_sum = own[i].sum()
    mean_pos_sim = pos_vals[i].sum() / (PER - 1)
    mean_neg_sim = (row_sum_excl_stripe + stripe_sum - own_sum) / (N - PER)

    return (np.float32(loss), np.float32(prec),
            np.float32(mean_pos_sim), np.float32(mean_neg_sim))


def kernel(inputs, targets):
    from concourse.bass_utils import run_bass_kernel_spmd

    perm, q_last, in_maps = _host_prep(inputs, targets)
    a_star = (q_last % RPC) // 128

    if a_star not in _compiled:
        _compiled[a_star] = _build_nc(a_star)
    nc = _compiled[a_star]

    res = run_bass_kernel_spmd(nc, in_maps, core_ids=list(range(NCORES)))
    return _assemble(res.results, perm, q_last)


# revision 8
# speedup vs baseline: 1.3180x; 1.3180x over previous
"""HardMiningLoss Trainium2 kernel.

Strategy (8 NeuronCores, data-parallel over anchor-row blocks):
  - Host: stable-sort rows by class (512 classes x 16 rows), cast to bf16.
    For core r, rotate rows by -1024*r so that core's 1024 anchors sit at
    column offset 0 -> the own-class 128-col "stripe" of local anchor tile a
    is always at columns [128a, 128a+128): fully static program, identical
    NEFF on all 8 cores, per-core behaviour comes only from input data.
  - Device per core: for each of 8 anchor tiles [128 x 8192]:
      PE   : 16 bf16 matmuls (fp32 PSUM) -> sim tile
      ACT  : PSUM -> SBUF evacuation with bf16 cast (4 x 2048-wide copies)
      DVE  : tiny stripe ops (save stripe, +/-1e30 block-diagonal masking,
             min_pos via tensor_scalar accum) then three full-row passes at
             4x bf16 mode, each a single tensor_scalar with accum_out:
               max_neg   = accum-max over masked row
               neg_cnt   = accum-sum of is_gt(sim, a_thr)
               summax    = accum-sum of max(sim, a_thr)
    Per-row stats + raw 128-wide stripes are DMA'd out.
  - Host: reconstructs neg_sum = summax - a_thr*(8192-cnt), computes the
    pos side (<=15 values/row, from the stripes), loss/prec/last-row means.
"""

import numpy as np
import ml_dtypes

N = 8192
D = 128
NCLS = 512
PER = 16            # rows per class (8192/512)
MARGIN = np.float32(0.1)
NCORES = 8
RPC = N // NCORES   # rows per core = 1024
TILES = RPC // 128  # anchor tiles per core = 8
BIG = np.float32(1e30)

_BF16 = ml_dtypes.bfloat16

_compiled = {}


def _build_nc(a_star: int):
    """Build the (identical-across-cores) bass program.

    No SBUF sim tile at all: the three reduction passes read PSUM chunks
    directly (DVE accum-max / DVE is_gt / ACT Sign / ACT Relu, all with
    accum_out), partials land in a [128,24] stats tile, host combines.
    Row sums come free from PE as an extra matvec against svec = sum_j x_j.
    a_star is unused (kept for signature compat)."""
    from contextlib import ExitStack
    import concourse.bacc as bacc
    import concourse.tile as tile
    import concourse.mybir as mybir

    dt = mybir.dt
    Alu = mybir.AluOpType
    Act = mybir.ActivationFunctionType

    nc = bacc.Bacc(
        "TRN2",
        debug=False,
        enable_asserts=False,
        target_bir_lowering=False,
        num_devices=NCORES,
    )

    xt_d = nc.dram_tensor("xt", [128, N], dt.bfloat16, kind="ExternalInput")
    sv_d = nc.dram_tensor("svec", [128, 1], dt.bfloat16, kind="ExternalInput")
    aneg_d = nc.dram_tensor("aneg", [128, 128], dt.bfloat16, kind="ExternalInput")
    apos_d = nc.dram_tensor("apos", [128, 128], dt.bfloat16, kind="ExternalInput")
    stats_d = nc.dram_tensor("stats", [TILES, 128, 24], dt.float32, kind="ExternalOutput")
    stripes_d = nc.dram_tensor("stripes", [TILES, 128, 128], dt.bfloat16, kind="ExternalOutput")

    with tile.TileContext(nc) as tc, ExitStack() as ctx:
        xtp = ctx.enter_context(tc.tile_pool(name="xtp", bufs=1))
        cstp = ctx.enter_context(tc.tile_pool(name="cstp", bufs=1))
        pp = ctx.enter_context(tc.tile_pool(name="pp", bufs=2, space="PSUM"))
        trp = ctx.enter_context(tc.tile_pool(name="trp", bufs=1))
        smp = ctx.enter_context(tc.tile_pool(name="smp", bufs=3))

        xt = xtp.tile([128, N], dt.bfloat16)
        for q in range(4):
            nc.sync.dma_start(out=xt[:, q * 2048:(q + 1) * 2048],
                              in_=xt_d[:, q * 2048:(q + 1) * 2048])
        aneg = cstp.tile([128, 128], dt.bfloat16)
        apos = cstp.tile([128, 128], dt.bfloat16)
        svec = cstp.tile([128, 1], dt.bfloat16)
        nc.sync.dma_start(out=aneg[:], in_=aneg_d[:, :])
        nc.sync.dma_start(out=apos[:], in_=apos_d[:, :])
        nc.sync.dma_start(out=svec[:], in_=sv_d[:, :])

        trd = trp.tile([128, 2048], dt.bfloat16)   # DVE trash
        tra = trp.tile([128, 2048], dt.bfloat16)   # ACT trash

        for a in range(TILES):
            lhsT = xt[:, a * 128:(a + 1) * 128]
            s0, s1 = a * 128, a * 128 + 128

            stats = smp.tile([128, 24], dt.float32, tag="stats")
            nc.vector.memset(stats[:], 0.0)
            nc.vector.memset(stats[:, 1:7], -1e30)

            athr = smp.tile([128, 1], dt.float32, tag="athr")
            nathr = smp.tile([128, 1], dt.float32, tag="nathr")
            ssv = smp.tile([128, 128], dt.bfloat16, tag="ssv")
            ssvn = smp.tile([128, 128], dt.bfloat16, tag="ssvn")
            spos = smp.tile([128, 128], dt.bfloat16, tag="spos")

            # partial col map: max 0L,0R,c1,c2,c3,stripe -> 1..6
            #                  sign 0L,0R,c1,c2 -> 7..10; isgt c3,stripe -> 11,12
            #                  relu 0L,0R,c1,c2,c3,stripe -> 13..18; rowsum -> 19
            for q in range(4):
                p = pp.tile([128, 2048], dt.float32, tag="ps")
                for c in range(4):
                    col = q * 2048 + c * 512
                    nc.tensor.matmul(p[:, c * 512:(c + 1) * 512], lhsT,
                                     xt[:, col:col + 512], start=True, stop=True)
                if q == 0:
                    # stripe chain
                    nc.vector.tensor_copy(ssv[:], p[:, s0:s1])
                    nc.sync.dma_start(out=stripes_d[a], in_=ssv[:])
                    nc.vector.tensor_tensor(ssvn[:], ssv[:], aneg[:], Alu.add)
                    nc.vector.tensor_tensor(spos[:], ssv[:], apos[:], Alu.add)
                    nc.vector.tensor_scalar(trd[:, :128], spos[:], 0.0, None,
                                            Alu.add, Alu.min,
                                            accum_out=stats[:, 0:1])
                    nc.vector.tensor_scalar_add(athr[:], stats[:, 0:1], -0.1)
                    nc.vector.tensor_scalar_mul(nathr[:], athr[:], -1.0)
                    # free row-sums via matvec into the dead stripe column
                    nc.tensor.matmul(p[:, s0:s0 + 1], lhsT, svec[:],
                                     start=True, stop=True)
                    nc.vector.tensor_copy(stats[:, 19:20], p[:, s0:s0 + 1])
                    regs = []
                    if a > 0:
                        regs.append((0, s0, 1, 7, 13))       # 0L
                    regs.append((s1, 2048, 2, 8, 14))        # 0R
                    # masked-stripe instrs (SBUF)
                    nc.vector.tensor_scalar(trd[:, :128], ssvn[:], 0.0, None,
                                            Alu.add, Alu.max,
                                            accum_out=stats[:, 6:7])
                    nc.vector.tensor_scalar(trd[:, :128], ssvn[:], athr[:], None,
                                            Alu.is_gt, Alu.add,
                                            accum_out=stats[:, 12:13])
                    nc.scalar.activation(tra[:, :128], ssvn[:], Act.Relu,
                                         bias=nathr[:], scale=1.0,
                                         accum_out=stats[:, 18:19])
                else:
                    regs = [(0, 2048, 2 + q, 8 + q if q < 3 else 11, 14 + q)]

                for lo, hi, kmax, kcnt, krelu in regs:
                    w = hi - lo
                    nc.vector.tensor_scalar(trd[:, :w], p[:, lo:hi], 0.0, None,
                                            Alu.add, Alu.max,
                                            accum_out=stats[:, kmax:kmax + 1])
                    if kcnt == 11:   # chunk3 counted on DVE
                        nc.vector.tensor_scalar(trd[:, :w], p[:, lo:hi],
                                                athr[:], None, Alu.is_gt, Alu.add,
                                                accum_out=stats[:, 11:12])
                    else:
                        nc.scalar.activation(tra[:, :w], p[:, lo:hi], Act.Sign,
                                             bias=nathr[:], scale=1.0,
                                             accum_out=stats[:, kcnt:kcnt + 1])
                    nc.scalar.activation(tra[:, :w], p[:, lo:hi], Act.Relu,
                                         bias=nathr[:], scale=1.0,
                                         accum_out=stats[:, krelu:krelu + 1])

            nc.sync.dma_start(out=stats_d[a], in_=stats[:])

    nc.compile()
    return nc


def _host_prep(inputs, targets):
    perm = np.argsort(targets, kind="stable")
    q_last = int(np.nonzero(perm == (N - 1))[0][0])
    Xs = np.asarray(inputs, dtype=np.float32)[perm]
    Xb = Xs.astype(_BF16)

    p = np.arange(128)
    blk_eq = (p[:, None] // PER) == (p[None, :] // PER)
    aneg = np.where(blk_eq, -BIG, np.float32(0)).astype(_BF16)
    apos = np.where(~blk_eq | np.eye(128, dtype=bool), BIG, np.float32(0)).astype(_BF16)

    svec_in = np.ascontiguousarray(
        Xb.astype(np.float32).sum(axis=0)[:, None].astype(_BF16))
    in_maps = []
    for r in range(NCORES):
        xrot = np.roll(Xb, -RPC * r, axis=0)
        in_maps.append({
            "xt": np.ascontiguousarray(xrot.T),
            "svec": svec_in,
            "aneg": aneg,
            "apos": apos,
        })
    return perm, q_last, in_maps


def _assemble(results, perm, q_last):
    """results: per-core dicts with 'stats' [8,128,24] f32, 'stripes'
    [8,128,128] bf16. Stats cols: 0 minpos | 1-6 max partials | 7-10 sign
    partials | 11 isgt chunk3 | 12 isgt stripe | 13-18 relu partials |
    19 rowsum(matvec)."""
    stats = np.stack([np.asarray(res["stats"], dtype=np.float32)
                      for res in results])
    stripes = np.stack([np.asarray(res["stripes"]).astype(np.float32)
                        for res in results])

    st = stats.reshape(N, 24)
    sv = stripes.reshape(N, 128)

    minpos = st[:, 0]
    maxneg = st[:, 1:7].max(axis=1)
    athr = minpos + np.float32(-0.1)

    SIGN_ELEMS = np.float32(6016.0)
    cnt = (st[:, 7:11].sum(axis=1) + SIGN_ELEMS) / 2 + st[:, 11] + st[:, 12]
    cnt = np.round(cnt)
    relusum = st[:, 13:19].sum(axis=1)
    neg_sum = relusum + athr * cnt
    neg_loss = neg_sum / np.maximum(cnt, 1.0)
    valid = cnt >= 1.0

    # pos side from stripes
    p = np.arange(N) % 128
    blk = (p // PER) * PER
    own = sv[np.arange(N)[:, None], blk[:, None] + np.arange(PER)[None, :]]
    self_idx = p % PER
    mask_self = np.ones((N, PER), dtype=bool)
    mask_self[np.arange(N), self_idx] = False
    pos_vals = own[mask_self].reshape(N, PER - 1)

    b = maxneg + MARGIN
    possel = pos_vals < b[:, None]
    pos_cnt = possel.sum(axis=1)
    pos_sum = np.where(possel, 1.0 - pos_vals, 0.0).sum(axis=1)
    pos_loss = pos_sum / np.maximum(pos_cnt, 1)

    loss = np.where(valid, pos_loss + neg_loss, 0.0).sum() / N
    prec = np.mean(1.0 - valid.astype(np.float32))

    i = q_last
    own_sum = own[i].sum()
    mean_pos_sim = pos_vals[i].sum() / (PER - 1)
    mean_neg_sim = (st[i, 19] - own_sum) / (N - PER)

    return (np.float32(loss), np.float32(prec),
            np.float32(mean_pos_sim), np.float32(mean_neg_sim))


def kernel(inputs, targets):
    from concourse.bass_utils import run_bass_kernel_spmd

    perm, q_last, in_maps = _host_prep(inputs, targets)
    a_star = (q_last % RPC) // 128

    if a_star not in _compiled:
        _compiled[a_star] = _build_nc(a_star)
    nc = _compiled[a_star]

    res = run_bass_kernel_spmd(nc, in_maps, core_ids=list(range(NCORES)))
    return _assemble(res.results, perm, q_last)
